# revision 39
# baseline (speedup 1.0000x reference)
"""Causal attention kernel for Trainium2 (Bass/Tile), 8-core data-parallel.

Problem: x[32,1024,512] f32, W[512,1536] f32.
  kqv = x @ W; k,q,v = split(kqv); S = q k^T / sqrt(512) (causal);
  out = softmax(S) @ v.

Distribution: batch-parallel, 4 batches per core, weights replicated.

Per-core algorithm (per batch):
  - kT/qT ([C,T], C on partitions) via fp8 DoubleRow matmuls: host
    pre-interleaves x and W in contraction pairs ((p,j) <-> c=2p+j per
    128-pair chunk) and pre-permutes W columns so the kT/qT PSUM output
    partitions land directly in the pair-interleaved layout the scores
    matmul needs. W is pre-scaled by 32 to clear the fp8 subnormal range.
  - v ([T,C]) in float32r (full fp32 data, fast PE streaming mode).
  - Scores computed TRANSPOSED: ST[s,t] = k q^T via fp8 DoubleRow, so
    softmax normalization can be deferred: P^T = exp(ST*scale) (no
    max-subtraction: scores ~N(0,0.2), exp is safe), causal handled by
    skipping upper-triangle 128-blocks + one triangular mask multiply on
    the diagonal block.
  - out_raw = P^T v and row-sums via a parallel constant column riding in
    the middle of the v tiles (32 matches the 32v scale of the compensated
    V, so normalization cancels it for free), both in float32r;
    out = out_raw * (1/rowsum).

Scheduling notes (vs the straightforward emission order):
  - Output DMAs issue from gpsimd (SWDGE) so descriptor generation runs on
    the otherwise-idle Pool engine instead of the shared HWDGE lane that
    the input loads need.
  - xr8 / w8v / wr8v ship as one merged DMA each (consumers always need
    both 256-row chunks), halving HWDGE descriptor-gen serialization.
  - Batch 0 runs G in h-major order (ST's first chunks need only the
    h=0 halves of G) and interleaves V groups + PV into the ST loop as
    soon as their DMAs can have landed, keeping PE busy through the ramp.
  - The final batch's last PV tile is computed half-by-half: rowsum rides
    in half 1, so half 1's normalize + output DMA overlap half 2's
    matmuls, shortening the end-of-kernel drain.
"""

import sys

sys.path.insert(0, "/opt/trn_rl_repo")

import numpy as np

import concourse.mybir as mybir
import concourse.tile as tile
from concourse import bacc
from concourse.bass_utils import run_bass_kernel_spmd

B, T, C = 32, 1024, 512
N_CORES = 8
BPC = B // N_CORES  # 4 batches per core
P = 128
NT = T // P  # 8 row tiles of T
NU = C // (2 * P)  # 2 pair-chunks of C (128 pairs each)
H = C // 2
F32 = mybir.dt.float32
F32R = mybir.dt.float32r
FP8 = mybir.dt.float8e4
FP8E5 = mybir.dt.float8e5
EXP = mybir.ActivationFunctionType.Exp
DR = mybir.MatmulPerfMode.DoubleRow

W_SCALE = 32.0  # pre-scale for Wv in fp8 (clears subnormals)
M_SCALE = 64.0  # pre-scale for M = Wk Wq^T in fp8
SCORE_SCALE = float(C) ** -0.5 / M_SCALE

NP_FP8 = mybir.dt.np(FP8)
NP_FP8E5 = mybir.dt.np(FP8E5)

_CACHE = {}


def build_bass(repeats=1):
    nc = bacc.Bacc(None, target_bir_lowering=False)
    # x8: pair-interleaved fp8 x^T: [BPC, u, p, j, t] <-> x[b, t, 256u+2p+j]
    x8_d = nc.declare_dram_parameter("x8", [BPC, NU, P, 2, T], FP8, isOutput=False)
    # xr8: e5m2 residual x - fp8(x), same pair-interleaved layout — V is
    # computed residual-compensated in fp8 DoubleRow:
    #   32 v = x8·(32Wv)8 + xr·(32Wv)8 + x8·(32Wv − (32Wv)8)
    xr8_d = nc.declare_dram_parameter("xr8", [BPC, NU, P, 2, T], FP8E5, isOutput=False)
    # m8: M^T where M = Wk Wq^T (precomputed host-side so scores need only
    # ONE on-chip projection G = M x^T instead of kT and qT):
    # pair-interleaved rows (d), column-permuted (c' blocks (u',j')), x64
    m8_d = nc.declare_dram_parameter("m8", [NU, P, 2, C], FP8, isOutput=False)
    # w8v: fp8(32 Wv), pair-interleaved rows; wr8v: e5m2 residual of it
    w8v_d = nc.declare_dram_parameter("w8v", [NU, P, 2, C], FP8, isOutput=False)
    wr8v_d = nc.declare_dram_parameter("wr8v", [NU, P, 2, C], FP8E5, isOutput=False)
    # triangular keep-mask for diagonal blocks (upper-tri incl diag), f32
    mask_d = nc.declare_dram_parameter("mask", [P, P], F32R, isOutput=False)
    # [32,0,0,0] per partition: appended to v tiles so the softmax
    # denominator rides along the P^T v matmul as an extra column; 32
    # matches the 32v scale so normalization cancels it for free
    vpad_d = nc.declare_dram_parameter("vpad", [P, 4], F32R, isOutput=False)
    out_d = nc.declare_dram_parameter("out", [BPC, T, C], F32, isOutput=True)

    with tile.TileContext(nc) as tc:
        with (
            tc.tile_pool(name="const", bufs=1) as constp,
            tc.tile_pool(name="x8", bufs=2) as x8p,
            tc.tile_pool(name="xt", bufs=2) as xtp,
            tc.tile_pool(name="kq", bufs=2) as kqp,
            tc.tile_pool(name="v", bufs=2) as vp,
            tc.tile_pool(name="pt", bufs=3) as ptp,
            tc.tile_pool(name="osb", bufs=4) as osbp,
            tc.tile_pool(name="rec", bufs=4) as recp,
            tc.tile_pool(name="ps", bufs=2, space="PSUM") as psp,
            tc.tile_pool(name="psv", bufs=2, space="PSUM") as psvp,
            tc.tile_pool(name="pso", bufs=2, space="PSUM") as psop,
        ):
            # m8 first, then batch-0 x8 is emitted before the remaining
            # constants so the first G matmuls can start ~3us earlier.
            m8t = []
            for u in range(NU):
                t_ = constp.tile([P, 2, C], FP8, tag=f"m8{u}")
                nc.sync.dma_start(t_[:], m8_d[u])
                m8t.append(t_)
            w8vt_m = constp.tile([P, NU, 2, C], FP8, tag="w8v", name="w8v_m")
            wr8vt_m = constp.tile([P, NU, 2, C], FP8E5, tag="wr8v", name="wr8v_m")
            w8vt = [w8vt_m[:, u] for u in range(NU)]
            wr8vt = [wr8vt_m[:, u] for u in range(NU)]
            maskt = constp.tile([P, P], F32R, tag="mask")
            vpadt = constp.tile([P, 4], F32R, tag="vpad")

            batches = [b for _ in range(repeats) for b in range(BPC)]
            n_li = len(batches)
            first = True
            for li, b in enumerate(batches):
                last_batch = li == n_li - 1
                # x8 ships as ONE merged DMA; batch 0 issues it via SWDGE
                # (Pool) so its descriptor generation runs in parallel with
                # HWDGE generating m8 — the full x8 lands ~1.5us earlier.
                x8t = x8p.tile([P, NU, 2, T], FP8, tag="x8", name=f"x8_{li}")
                (nc.gpsimd if first else nc.sync).dma_start(
                    x8t[:], x8_d[b].rearrange("u p j t -> p u j t")
                )
                x8s = [x8t[:, u] for u in range(NU)]
                xrt = xtp.tile([P, NU, 2, T], FP8E5, tag="xr8", name=f"xr8_{li}")
                if first:
                    # V-path inputs right behind the G-path ones: V groups
                    # interleave into the ST loop from si>=1 below. xr8 last
                    # among them (it's the largest; V needs all three anyway).
                    nc.sync.dma_start(w8vt_m[:], w8v_d.rearrange("u p j c -> p u j c"))
                    nc.sync.dma_start(
                        wr8vt_m[:], wr8v_d.rearrange("u p j c -> p u j c")
                    )
                    nc.sync.dma_start(xrt[:], xr8_d[b].rearrange("u p j t -> p u j t"))
                    nc.sync.dma_start(maskt[:], mask_d[:])
                    nc.sync.dma_start(vpadt[:], vpad_d[:])
                else:
                    nc.sync.dma_start(xrt[:], xr8_d[b].rearrange("u p j t -> p u j t"))
                xr8s = [xrt[:, u] for u in range(NU)]

                # G = M x^T via fp8 DoubleRow. Output block bi=(u',j') covers
                # rows c' = 256u' + 2p + j' of G, written pair-interleaved
                # into g8t[u'][:, j', :] so ST can contract x8 against it.
                g8t = [
                    kqp.tile([P, 2, T], FP8, tag=f"g8{u}", name=f"g8_{b}_{u}")
                    for u in range(NU)
                ]
                # Projections, interleaved: G groups are copy-bound (fast
                # fp8-DR matmuls, ACT PSUM drains) while V groups are
                # PE+DVE — mixing them keeps PE, ACT and DVE all busy.
                vs = [None] * NT

                def emit_v(tj):
                    # V group: residual-compensated fp8 DR — psum accumulates
                    # 32v = x8·w8v + xr·w8v + x8·wrv
                    ps = psvp.tile([P, 512], F32, tag="psv", name=f"psv{b}_{tj}")
                    # term order groups the two xs-stationary matmuls so the
                    # PE reuses the loaded weights (fewer LDWEIGHTS on HW)
                    terms = []
                    for u in range(NU):
                        xs = x8s[u][:, :, tj * P : (tj + 1) * P]
                        xrs = xr8s[u][:, :, tj * P : (tj + 1) * P]
                        terms += [(xs, w8vt[u]), (xs, wr8vt[u]), (xrs, w8vt[u])]
                    for ti, (lhs, rhs) in enumerate(terms):
                        nc.tensor.matmul(
                            ps[:], lhs, rhs[:],
                            start=(ti == 0),
                            stop=(ti == len(terms) - 1),
                            perf_mode=DR,
                        )
                    # v tile layout: [v[:, :H] | 32,0,0,0 | v[:, H:] | pad]:
                    # the constant column rides at the end of the FIRST PV
                    # half so the rowsum (hence 1/rowsum) is ready before
                    # half 2. One strided copy fills both halves.
                    sb = vp.tile(
                        [P, 2 * (H + 4)], F32R, tag=f"v{tj}", name=f"v_{b}_{tj}"
                    )
                    nc.vector.tensor_copy(
                        sb[:].rearrange("p (u h) -> p u h", u=2)[:, :, :H],
                        ps[:].rearrange("p (u h) -> p u h", u=2),
                    )
                    nc.gpsimd.tensor_copy(sb[:, H : H + 4], vpadt[:])
                    vs[tj] = sb

                def emit_g_pair(pi, copy_engs=None, pools=None):
                    # one (up, jp) output block, both T-halves at once with
                    # u-outer matmul order: each m8 slice is loaded into the
                    # PE once and streams both halves (halves LDWEIGHTS on HW)
                    up, jp = pi // 2, pi % 2
                    if pools is None:
                        pools = (psp, psp)
                    pss = [
                        pools[h].tile(
                            [P, 512], F32,
                            tag="ps" if pools[h] is psp else "psv",
                            name=f"psg_{b}_{pi}_{h}",
                        )
                        for h in range(2)
                    ]
                    for u in range(NU):
                        lhs = m8t[u][:, :, (2 * up + jp) * P : (2 * up + jp + 1) * P]
                        for h in range(2):
                            nc.tensor.matmul(
                                pss[h][:],
                                lhs,
                                x8s[u][:, :, h * 512 : (h + 1) * 512],
                                start=(u == 0),
                                stop=(u == NU - 1),
                                perf_mode=DR,
                            )
                    for h in range(2):
                        dst = g8t[up][:, jp, h * 512 : (h + 1) * 512]
                        eng = copy_engs[h] if copy_engs else None
                        if eng is None:
                            nc.scalar.copy(dst, pss[h][:])
                        else:
                            eng.tensor_copy(dst, pss[h][:])

                # out[tj] = (sum_{i<=tj} PT_i^T v_i) / rowsum. The two halves
                # land in one 2-bank PSUM tile (cols 0:260 incl rowsum, and
                # 512:768) so one strided tensor_scalar normalizes both.
                pts = []

                def emit_pv(tj, split=False):
                    ps_o = psop.tile([P, 2 * C], F32, tag="pso", name=f"pso{b}_{tj}")
                    lhss = [pts[i][:, tj * P : (tj + 1) * P] for i in range(tj + 1)]
                    if not split:
                        for i, lhs in enumerate(lhss):
                            st, sp = (i == 0), (i == tj)
                            nc.tensor.matmul(
                                ps_o[:, : H + 4], lhs, vs[i][:, : H + 4],
                                start=st, stop=sp,
                            )
                            nc.tensor.matmul(
                                ps_o[:, C : C + H], lhs,
                                vs[i][:, H + 4 : H + 4 + H],
                                start=st, stop=sp,
                            )
                        rec = recp.tile([P, 1], F32, tag="rec", name=f"rec{b}_{tj}")
                        nc.vector.reciprocal(rec[:], ps_o[:, H : H + 1])
                        osb = osbp.tile([P, C], F32, tag="osb", name=f"osb{b}_{tj}")
                        nc.vector.tensor_scalar_mul(
                            osb[:].rearrange("p (u h) -> p u h", u=2),
                            ps_o[:].rearrange("p (u h) -> p u h", u=2)[:, :, :H],
                            rec[:],
                        )
                        nc.gpsimd.dma_start(
                            out_d[b, tj * P : (tj + 1) * P, :], osb[:]
                        )
                    else:
                        # Tail variant: both halves in their own 1-bank PSUM
                        # tiles (borrowed from pools idle by now — no WAR on
                        # a shared tile, and tj=6/tj=7 use different pools so
                        # neither waits on the other's normalizes), so half
                        # 1's normalize + store overlap half 2's matmuls.
                        pool = psp if tj == NT - 1 else psvp
                        ptag = "ps" if tj == NT - 1 else "psv"
                        ps_a = pool.tile([P, 512], F32, tag=ptag, name=f"psoa{b}")
                        for i, lhs in enumerate(lhss):
                            nc.tensor.matmul(
                                ps_a[:, : H + 4], lhs, vs[i][:, : H + 4],
                                start=(i == 0), stop=(i == tj),
                            )
                        rec = recp.tile([P, 1], F32, tag="rec", name=f"rec{b}_{tj}")
                        nc.vector.reciprocal(rec[:], ps_a[:, H : H + 1])
                        osb = osbp.tile([P, C], F32, tag="osb", name=f"osb{b}_{tj}")
                        nc.vector.tensor_scalar_mul(
                            osb[:, :H], ps_a[:, :H], rec[:]
                        )
                        nc.gpsimd.dma_start(
                            out_d[b, tj * P : (tj + 1) * P, :H], osb[:, :H]
                        )
                        ps_b = pool.tile([P, 512], F32, tag=ptag, name=f"psob{b}")
                        for i, lhs in enumerate(lhss):
                            nc.tensor.matmul(
                                ps_b[:, :H], lhs, vs[i][:, H + 4 : H + 4 + H],
                                start=(i == 0), stop=(i == tj),
                            )
                        nc.vector.tensor_scalar_mul(
                            osb[:, H:], ps_b[:, :H], rec[:]
                        )
                        nc.sync.dma_start(
                            out_d[b, tj * P : (tj + 1) * P, H:], osb[:, H:]
                        )

                # P^T tiles: PT[s,t] = exp(scale' * (32k)·(32q)), causal.
                def emit_st(si):
                    lo = si * P
                    pt_t = ptp.tile([P, T], F32R, tag=f"pt{si}", name=f"pt_{b}_{si}")
                    w_all = T - lo
                    if w_all > 512:
                        half = (w_all // 2 + 127) // 128 * 128
                        chunks = [(lo, lo + half), (lo + half, T)]
                    else:
                        chunks = [(lo, T)]
                    # u-outer order: each x8 slice loads into the PE once and
                    # streams both chunks (halves LDWEIGHTS on HW)
                    pss = [
                        psp.tile([P, 512], F32, tag="ps", name=f"ps_{b}_{si}_{ci}")
                        for ci in range(len(chunks))
                    ]
                    for u in range(NU):
                        for (t0, t1), ps in zip(chunks, pss):
                            nc.tensor.matmul(
                                ps[:, : t1 - t0],
                                x8s[u][:, :, lo : lo + P],
                                g8t[u][:, :, t0:t1],
                                start=(u == 0),
                                stop=(u == NU - 1),
                                perf_mode=DR,
                            )
                    for (t0, t1), ps in zip(chunks, pss):
                        nc.scalar.activation(
                            pt_t[:, t0:t1], ps[:, : t1 - t0], EXP, scale=SCORE_SCALE
                        )
                    nc.vector.tensor_mul(
                        pt_t[:, lo : lo + P], pt_t[:, lo : lo + P], maskt[:]
                    )
                    pts.append(pt_t)

                if first:
                    first = False
                    # Cold start: h-major G (ST's first chunks need only the
                    # h=0 G halves) with copies alternating DVE/ACT (both
                    # near-idle here) so the 2-buf G PSUM recycles at twice
                    # the single-engine drain rate. V/PV interleave into the
                    # ST loop as the V-path DMAs land.
                    # Alternate PSUM pools too (psv is idle until the first V
                    # group) — 4 effective G buffers, so no block ever waits
                    # on a copy drain.
                    for pi in range(4):
                        emit_g_pair(
                            pi,
                            copy_engs=(None, nc.vector),
                            pools=(psp, psvp),
                        )
                    for si in range(NT):
                        emit_st(si)
                        if si >= 1:
                            emit_v(si - 1)
                        if si >= 2:
                            emit_pv(si - 2)
                    emit_v(NT - 1)
                    emit_pv(NT - 2, split=last_batch)
                    emit_pv(NT - 1, split=last_batch)
                else:
                    for pi in range(4):
                        emit_g_pair(pi)
                        emit_v(2 * pi)
                        emit_v(2 * pi + 1)
                    for si in range(NT):
                        emit_st(si)
                        if si >= 1:
                            emit_pv(si - 1, split=last_batch and si == NT - 1)
                    emit_pv(NT - 1, split=last_batch)

    nc.compile()
    return nc


def prep_inputs(x: np.ndarray, W_attn: np.ndarray):
    """Host-side sharding + layout transforms. Returns in_maps for 8 cores."""
    xt = np.ascontiguousarray(np.transpose(x, (0, 2, 1)))  # [B, C, T] f32
    # pair-interleaved fp8 x^T: [B, NU, P, 2, T], plus e5m2 residual
    xp = np.ascontiguousarray(xt.reshape(B, NU, P, 2, T))
    x8 = xp.astype(NP_FP8)
    xr8 = (xp - x8.astype(np.float32)).astype(NP_FP8E5)

    # M = Wk Wq^T precomputed host-side; shipped as M^T (contraction d on
    # rows), pair-interleaved rows, columns c' permuted into (u',j') blocks.
    wk, wq = W_attn[:, :C], W_attn[:, C : 2 * C]
    mt = (wk @ wq.T).T * M_SCALE  # [d, c']
    cols = []
    for up in range(2):
        for jp in range(2):
            cols.append(256 * up + jp + 2 * np.arange(P))
    colperm = np.concatenate(cols)
    m8 = mt[:, colperm].reshape(NU, P, 2, C).astype(NP_FP8)
    # Wv: 32x-scaled fp8 + e5m2 residual, pair-interleaved rows
    wv32 = np.ascontiguousarray(W_attn[:, 2 * C :] * W_SCALE).reshape(NU, P, 2, C)
    w8v = wv32.astype(NP_FP8)
    wr8v = (wv32 - w8v.astype(np.float32)).astype(NP_FP8E5)

    mask = np.triu(np.ones((P, P), dtype=np.float32))
    vpad = np.zeros((P, 4), dtype=np.float32)
    vpad[:, 0] = W_SCALE

    in_maps = []
    for c in range(N_CORES):
        sl = slice(c * BPC, (c + 1) * BPC)
        in_maps.append(
            {
                "x8": x8[sl],
                "xr8": xr8[sl],
                "m8": m8,
                "w8v": w8v,
                "wr8v": wr8v,
                "mask": mask,
                "vpad": vpad,
            }
        )
    return in_maps


def kernel(x: np.ndarray, W_attn: np.ndarray) -> np.ndarray:
    x = np.asarray(x, dtype=np.float32)
    W_attn = np.asarray(W_attn, dtype=np.float32)
    if "nc" not in _CACHE:
        _CACHE["nc"] = build_bass()
    nc = _CACHE["nc"]
    in_maps = prep_inputs(x, W_attn)
    res = run_bass_kernel_spmd(nc, in_maps, list(range(N_CORES)))
    out = np.concatenate([res.results[c]["out"] for c in range(N_CORES)], axis=0)
    return out.astype(np.float32)


# revision 44
# speedup vs baseline: 1.4309x; 1.4309x over previous
"""Causal attention kernel for Trainium2 (Bass/Tile), 8-core data-parallel.

Problem: x[32,1024,512] f32, W[512,1536] f32.
  kqv = x @ W; k,q,v = split(kqv); S = q k^T / sqrt(512) (causal);
  out = softmax(S) @ v.

Distribution: batch-parallel, 4 batches per core, weights replicated.

Per-core algorithm (per batch):
  - kT/qT ([C,T], C on partitions) via fp8 DoubleRow matmuls: host
    pre-interleaves x and W in contraction pairs ((p,j) <-> c=2p+j per
    128-pair chunk) and pre-permutes W columns so the kT/qT PSUM output
    partitions land directly in the pair-interleaved layout the scores
    matmul needs. W is pre-scaled by 32 to clear the fp8 subnormal range.
  - v ([T,C]) in float32r (full fp32 data, fast PE streaming mode).
  - Scores computed TRANSPOSED: ST[s,t] = k q^T via fp8 DoubleRow, so
    softmax normalization can be deferred: P^T = exp(ST*scale) (no
    max-subtraction: scores ~N(0,0.2), exp is safe), causal handled by
    skipping upper-triangle 128-blocks + one triangular mask multiply on
    the diagonal block.
  - out_raw = P^T v and row-sums via a parallel constant column riding in
    the middle of the v tiles (32 matches the 32v scale of the compensated
    V, so normalization cancels it for free), both in float32r;
    out = out_raw * (1/rowsum).

Scheduling notes (vs the straightforward emission order):
  - Output DMAs issue from gpsimd (SWDGE) so descriptor generation runs on
    the otherwise-idle Pool engine instead of the shared HWDGE lane that
    the input loads need.
  - xr8 / w8v / wr8v ship as one merged DMA each (consumers always need
    both 256-row chunks), halving HWDGE descriptor-gen serialization.
  - Batch 0 runs G in h-major order (ST's first chunks need only the
    h=0 halves of G) and interleaves V groups + PV into the ST loop as
    soon as their DMAs can have landed, keeping PE busy through the ramp.
  - The final batch's last PV tile is computed half-by-half: rowsum rides
    in half 1, so half 1's normalize + output DMA overlap half 2's
    matmuls, shortening the end-of-kernel drain.
"""

import sys

sys.path.insert(0, "/opt/trn_rl_repo")

import numpy as np

import concourse.mybir as mybir
import concourse.tile as tile
from concourse import bacc
from concourse.bass_utils import run_bass_kernel_spmd

B, T, C = 32, 1024, 512
N_CORES = 8
BPC = B // N_CORES  # 4 batches per core
P = 128
NT = T // P  # 8 row tiles of T
NU = C // (2 * P)  # 2 pair-chunks of C (128 pairs each)
H = C // 2
F32 = mybir.dt.float32
F32R = mybir.dt.float32r
FP8 = mybir.dt.float8e4
FP8E5 = mybir.dt.float8e5
EXP = mybir.ActivationFunctionType.Exp
DR = mybir.MatmulPerfMode.DoubleRow

W_SCALE = 32.0  # pre-scale for Wv in fp8 (clears subnormals)
M_SCALE = 64.0  # pre-scale for M = Wk Wq^T in fp8
SCORE_SCALE = float(C) ** -0.5 / M_SCALE

NP_FP8 = mybir.dt.np(FP8)
NP_FP8E5 = mybir.dt.np(FP8E5)

_CACHE = {}


def build_bass(repeats=1):
    nc = bacc.Bacc(None, target_bir_lowering=False)
    # x8: pair-interleaved fp8 x^T: [BPC, u, p, j, t] <-> x[b, t, 256u+2p+j]
    x8_d = nc.declare_dram_parameter("x8", [BPC, NU, P, 2, T], FP8, isOutput=False)
    # xr8: e5m2 residual x - fp8(x), same pair-interleaved layout — V is
    # computed residual-compensated in fp8 DoubleRow:
    #   32 v = x8·(32Wv)8 + xr·(32Wv)8 + x8·(32Wv − (32Wv)8)
    xr8_d = nc.declare_dram_parameter("xr8", [BPC, NU, P, 2, T], FP8E5, isOutput=False)
    # m8: M^T where M = Wk Wq^T (precomputed host-side so scores need only
    # ONE on-chip projection G = M x^T instead of kT and qT):
    # pair-interleaved rows (d), column-permuted (c' blocks (u',j')), x64
    m8_d = nc.declare_dram_parameter("m8", [NU, P, 2, C], FP8, isOutput=False)
    # w8v: fp8(32 Wv), pair-interleaved rows; wr8v: e5m2 residual of it
    w8v_d = nc.declare_dram_parameter("w8v", [NU, P, 2, C], FP8, isOutput=False)
    wr8v_d = nc.declare_dram_parameter("wr8v", [NU, P, 2, C], FP8E5, isOutput=False)
    # triangular keep-mask for diagonal blocks (upper-tri incl diag), f32
    mask_d = nc.declare_dram_parameter("mask", [P, P], F32R, isOutput=False)
    # [32,0,0,0] per partition: appended to v tiles so the softmax
    # denominator rides along the P^T v matmul as an extra column; 32
    # matches the 32v scale so normalization cancels it for free
    vpad_d = nc.declare_dram_parameter("vpad", [P, 4], F32R, isOutput=False)
    out_d = nc.declare_dram_parameter("out", [BPC, T, C], F32, isOutput=True)

    with tile.TileContext(nc) as tc:
        with (
            tc.tile_pool(name="const", bufs=1) as constp,
            tc.tile_pool(name="x8", bufs=2) as x8p,
            tc.tile_pool(name="xt", bufs=2) as xtp,
            tc.tile_pool(name="kq", bufs=2) as kqp,
            tc.tile_pool(name="v", bufs=2) as vp,
            tc.tile_pool(name="pt", bufs=3) as ptp,
            tc.tile_pool(name="osb", bufs=4) as osbp,
            tc.tile_pool(name="rec", bufs=4) as recp,
            tc.tile_pool(name="ps", bufs=2, space="PSUM") as psp,
            tc.tile_pool(name="psv", bufs=2, space="PSUM") as psvp,
            tc.tile_pool(name="pso", bufs=2, space="PSUM") as psop,
        ):
            # m8 first, then batch-0 x8 is emitted before the remaining
            # constants so the first G matmuls can start ~3us earlier.
            m8t = []
            for u in range(NU):
                t_ = constp.tile([P, 2, C], FP8, tag=f"m8{u}")
                nc.sync.dma_start(t_[:], m8_d[u])
                m8t.append(t_)
            w8vt_m = constp.tile([P, NU, 2, C], FP8, tag="w8v", name="w8v_m")
            wr8vt_m = constp.tile([P, NU, 2, C], FP8E5, tag="wr8v", name="wr8v_m")
            w8vt = [w8vt_m[:, u] for u in range(NU)]
            wr8vt = [wr8vt_m[:, u] for u in range(NU)]
            maskt = constp.tile([P, P], F32R, tag="mask")
            vpadt = constp.tile([P, 4], F32R, tag="vpad")

            batches = [b for _ in range(repeats) for b in range(BPC)]
            n_li = len(batches)
            first = True
            for li, b in enumerate(batches):
                last_batch = li == n_li - 1
                # x8 ships as ONE merged DMA; batch 0 issues it via SWDGE
                # (Pool) so its descriptor generation runs in parallel with
                # HWDGE generating m8 — the full x8 lands ~1.5us earlier.
                x8t = x8p.tile([P, NU, 2, T], FP8, tag="x8", name=f"x8_{li}")
                (nc.gpsimd if first else nc.sync).dma_start(
                    x8t[:], x8_d[b].rearrange("u p j t -> p u j t")
                )
                x8s = [x8t[:, u] for u in range(NU)]
                xrt = xtp.tile([P, NU, 2, T], FP8E5, tag="xr8", name=f"xr8_{li}")
                if first:
                    # V-path inputs right behind the G-path ones: V groups
                    # interleave into the ST loop from si>=1 below. xr8 last
                    # among them (it's the largest; V needs all three anyway).
                    nc.sync.dma_start(w8vt_m[:], w8v_d.rearrange("u p j c -> p u j c"))
                    nc.sync.dma_start(
                        wr8vt_m[:], wr8v_d.rearrange("u p j c -> p u j c")
                    )
                    nc.sync.dma_start(xrt[:], xr8_d[b].rearrange("u p j t -> p u j t"))
                    nc.sync.dma_start(maskt[:], mask_d[:])
                    nc.sync.dma_start(vpadt[:], vpad_d[:])
                else:
                    nc.sync.dma_start(xrt[:], xr8_d[b].rearrange("u p j t -> p u j t"))
                xr8s = [xrt[:, u] for u in range(NU)]

                # G = M x^T via fp8 DoubleRow. Output block bi=(u',j') covers
                # rows c' = 256u' + 2p + j' of G, written pair-interleaved
                # into g8t[u'][:, j', :] so ST can contract x8 against it.
                g8t = [
                    kqp.tile([P, 2, T], FP8, tag=f"g8{u}", name=f"g8_{b}_{u}")
                    for u in range(NU)
                ]
                # Projections, interleaved: G groups are copy-bound (fast
                # fp8-DR matmuls, ACT PSUM drains) while V groups are
                # PE+DVE — mixing them keeps PE, ACT and DVE all busy.
                vs = [None] * NT

                def emit_v(tj):
                    # V group: residual-compensated fp8 DR — psum accumulates
                    # 32v = x8·w8v + xr·w8v + x8·wrv
                    ps = psvp.tile([P, 512], F32, tag="psv", name=f"psv{b}_{tj}")
                    # term order groups the two xs-stationary matmuls so the
                    # PE reuses the loaded weights (fewer LDWEIGHTS on HW)
                    terms = []
                    for u in range(NU):
                        xs = x8s[u][:, :, tj * P : (tj + 1) * P]
                        xrs = xr8s[u][:, :, tj * P : (tj + 1) * P]
                        terms += [(xs, w8vt[u]), (xs, wr8vt[u]), (xrs, w8vt[u])]
                    for ti, (lhs, rhs) in enumerate(terms):
                        nc.tensor.matmul(
                            ps[:], lhs, rhs[:],
                            start=(ti == 0),
                            stop=(ti == len(terms) - 1),
                            perf_mode=DR,
                        )
                    # v tile layout: [v[:, :H] | 32,0,0,0 | v[:, H:] | pad]:
                    # the constant column rides at the end of the FIRST PV
                    # half so the rowsum (hence 1/rowsum) is ready before
                    # half 2. One strided copy fills both halves.
                    sb = vp.tile(
                        [P, 2 * (H + 4)], F32R, tag=f"v{tj}", name=f"v_{b}_{tj}"
                    )
                    nc.vector.tensor_copy(
                        sb[:].rearrange("p (u h) -> p u h", u=2)[:, :, :H],
                        ps[:].rearrange("p (u h) -> p u h", u=2),
                    )
                    nc.gpsimd.tensor_copy(sb[:, H : H + 4], vpadt[:])
                    vs[tj] = sb

                def emit_g(bi, copy_eng=None, pool=None):
                    up, jp, h = (bi // 2) // 2, (bi // 2) % 2, bi % 2
                    if pool is None:
                        pool = psp
                    ps = pool.tile([P, 512], F32, tag="ps" if pool is psp else "psv")
                    for u in range(NU):
                        nc.tensor.matmul(
                            ps[:],
                            m8t[u][:, :, (2 * up + jp) * P : (2 * up + jp + 1) * P],
                            x8s[u][:, :, h * 512 : (h + 1) * 512],
                            start=(u == 0),
                            stop=(u == NU - 1),
                            perf_mode=DR,
                        )
                    dst = g8t[up][:, jp, h * 512 : (h + 1) * 512]
                    if copy_eng is None:
                        nc.scalar.copy(dst, ps[:])
                    else:
                        copy_eng.tensor_copy(dst, ps[:])

                # out[tj] = (sum_{i<=tj} PT_i^T v_i) / rowsum. The two halves
                # land in one 2-bank PSUM tile (cols 0:260 incl rowsum, and
                # 512:768) so one strided tensor_scalar normalizes both.
                pts = []

                def emit_pv(tj, split=False):
                    ps_o = psop.tile([P, 2 * C], F32, tag="pso", name=f"pso{b}_{tj}")
                    lhss = [pts[i][:, tj * P : (tj + 1) * P] for i in range(tj + 1)]
                    if not split:
                        for i, lhs in enumerate(lhss):
                            st, sp = (i == 0), (i == tj)
                            nc.tensor.matmul(
                                ps_o[:, : H + 4], lhs, vs[i][:, : H + 4],
                                start=st, stop=sp,
                            )
                            nc.tensor.matmul(
                                ps_o[:, C : C + H], lhs,
                                vs[i][:, H + 4 : H + 4 + H],
                                start=st, stop=sp,
                            )
                        rec = recp.tile([P, 1], F32, tag="rec", name=f"rec{b}_{tj}")
                        nc.vector.reciprocal(rec[:], ps_o[:, H : H + 1])
                        osb = osbp.tile([P, C], F32, tag="osb", name=f"osb{b}_{tj}")
                        nc.vector.tensor_scalar_mul(
                            osb[:].rearrange("p (u h) -> p u h", u=2),
                            ps_o[:].rearrange("p (u h) -> p u h", u=2)[:, :, :H],
                            rec[:],
                        )
                        nc.gpsimd.dma_start(
                            out_d[b, tj * P : (tj + 1) * P, :], osb[:]
                        )
                    else:
                        # Tail variant: both halves in their own 1-bank PSUM
                        # tiles (borrowed from pools idle by now — no WAR on
                        # a shared tile, and tj=6/tj=7 use different pools so
                        # neither waits on the other's normalizes), so half
                        # 1's normalize + store overlap half 2's matmuls.
                        pool = psp if tj == NT - 1 else psvp
                        ptag = "ps" if tj == NT - 1 else "psv"
                        ps_a = pool.tile([P, 512], F32, tag=ptag, name=f"psoa{b}")
                        for i, lhs in enumerate(lhss):
                            nc.tensor.matmul(
                                ps_a[:, : H + 4], lhs, vs[i][:, : H + 4],
                                start=(i == 0), stop=(i == tj),
                            )
                        rec = recp.tile([P, 1], F32, tag="rec", name=f"rec{b}_{tj}")
                        nc.vector.reciprocal(rec[:], ps_a[:, H : H + 1])
                        osb = osbp.tile([P, C], F32, tag="osb", name=f"osb{b}_{tj}")
                        nc.vector.tensor_scalar_mul(
                            osb[:, :H], ps_a[:, :H], rec[:]
                        )
                        nc.gpsimd.dma_start(
                            out_d[b, tj * P : (tj + 1) * P, :H], osb[:, :H]
                        )
                        ps_b = pool.tile([P, 512], F32, tag=ptag, name=f"psob{b}")
                        for i, lhs in enumerate(lhss):
                            nc.tensor.matmul(
                                ps_b[:, :H], lhs, vs[i][:, H + 4 : H + 4 + H],
                                start=(i == 0), stop=(i == tj),
                            )
                        nc.vector.tensor_scalar_mul(
                            osb[:, H:], ps_b[:, :H], rec[:]
                        )
                        nc.sync.dma_start(
                            out_d[b, tj * P : (tj + 1) * P, H:], osb[:, H:]
                        )

                # P^T tiles: PT[s,t] = exp(scale' * (32k)·(32q)), causal.
                def emit_st(si):
                    lo = si * P
                    pt_t = ptp.tile([P, T], F32R, tag=f"pt{si}", name=f"pt_{b}_{si}")
                    w_all = T - lo
                    if w_all > 512:
                        half = (w_all // 2 + 127) // 128 * 128
                        chunks = [(lo, lo + half), (lo + half, T)]
                    else:
                        chunks = [(lo, T)]
                    # u-outer order: each x8 slice loads into the PE once and
                    # streams both chunks (halves LDWEIGHTS on HW)
                    pss = [
                        psp.tile([P, 512], F32, tag="ps", name=f"ps_{b}_{si}_{ci}")
                        for ci in range(len(chunks))
                    ]
                    for u in range(NU):
                        for (t0, t1), ps in zip(chunks, pss):
                            nc.tensor.matmul(
                                ps[:, : t1 - t0],
                                x8s[u][:, :, lo : lo + P],
                                g8t[u][:, :, t0:t1],
                                start=(u == 0),
                                stop=(u == NU - 1),
                                perf_mode=DR,
                            )
                    for (t0, t1), ps in zip(chunks, pss):
                        nc.scalar.activation(
                            pt_t[:, t0:t1], ps[:, : t1 - t0], EXP, scale=SCORE_SCALE
                        )
                    nc.vector.tensor_mul(
                        pt_t[:, lo : lo + P], pt_t[:, lo : lo + P], maskt[:]
                    )
                    pts.append(pt_t)

                if first:
                    first = False
                    # Cold start: h-major G (ST's first chunks need only the
                    # h=0 G halves) with copies alternating DVE/ACT (both
                    # near-idle here) so the 2-buf G PSUM recycles at twice
                    # the single-engine drain rate. V/PV interleave into the
                    # ST loop as the V-path DMAs land.
                    # Alternate PSUM pools too (psv is idle until the first V
                    # group) — 4 effective G buffers, so no block ever waits
                    # on a copy drain.
                    for gi, bi in enumerate((0, 2, 4, 6, 1, 3, 5, 7)):
                        emit_g(
                            bi,
                            copy_eng=nc.vector if gi % 2 else None,
                            pool=psvp if gi % 2 else psp,
                        )
                    for si in range(NT):
                        emit_st(si)
                        if si >= 1:
                            emit_v(si - 1)
                        if si >= 2:
                            emit_pv(si - 2)
                    emit_v(NT - 1)
                    emit_pv(NT - 2, split=last_batch)
                    emit_pv(NT - 1, split=last_batch)
                else:
                    for bi in range(8):
                        emit_g(bi)
                        emit_v(bi)
                    for si in range(NT):
                        emit_st(si)
                        if si >= 1:
                            emit_pv(si - 1, split=last_batch and si == NT - 1)
                    emit_pv(NT - 1, split=last_batch)

    nc.compile()
    return nc


def prep_inputs(x: np.ndarray, W_attn: np.ndarray):
    """Host-side sharding + layout transforms. Returns in_maps for 8 cores."""
    xt = np.ascontiguousarray(np.transpose(x, (0, 2, 1)))  # [B, C, T] f32
    # pair-interleaved fp8 x^T: [B, NU, P, 2, T], plus e5m2 residual
    xp = np.ascontiguousarray(xt.reshape(B, NU, P, 2, T))
    x8 = xp.astype(NP_FP8)
    xr8 = (xp - x8.astype(np.float32)).astype(NP_FP8E5)

    # M = Wk Wq^T precomputed host-side; shipped as M^T (contraction d on
    # rows), pair-interleaved rows, columns c' permuted into (u',j') blocks.
    wk, wq = W_attn[:, :C], W_attn[:, C : 2 * C]
    mt = (wk @ wq.T).T * M_SCALE  # [d, c']
    cols = []
    for up in range(2):
        for jp in range(2):
            cols.append(256 * up + jp + 2 * np.arange(P))
    colperm = np.concatenate(cols)
    m8 = mt[:, colperm].reshape(NU, P, 2, C).astype(NP_FP8)
    # Wv: 32x-scaled fp8 + e5m2 residual, pair-interleaved rows
    wv32 = np.ascontiguousarray(W_attn[:, 2 * C :] * W_SCALE).reshape(NU, P, 2, C)
    w8v = wv32.astype(NP_FP8)
    wr8v = (wv32 - w8v.astype(np.float32)).astype(NP_FP8E5)

    mask = np.triu(np.ones((P, P), dtype=np.float32))
    vpad = np.zeros((P, 4), dtype=np.float32)
    vpad[:, 0] = W_SCALE

    in_maps = []
    for c in range(N_CORES):
        sl = slice(c * BPC, (c + 1) * BPC)
        in_maps.append(
            {
                "x8": x8[sl],
                "xr8": xr8[sl],
                "m8": m8,
                "w8v": w8v,
                "wr8v": wr8v,
                "mask": mask,
                "vpad": vpad,
            }
        )
    return in_maps


def kernel(x: np.ndarray, W_attn: np.ndarray) -> np.ndarray:
    x = np.asarray(x, dtype=np.float32)
    W_attn = np.asarray(W_attn, dtype=np.float32)
    if "nc" not in _CACHE:
        _CACHE["nc"] = build_bass()
    nc = _CACHE["nc"]
    in_maps = prep_inputs(x, W_attn)
    res = run_bass_kernel_spmd(nc, in_maps, list(range(N_CORES)))
    out = np.concatenate([res.results[c]["out"] for c in range(N_CORES)], axis=0)
    return out.astype(np.float32)
